# revision 21
# baseline (speedup 1.0000x reference)
"""Multi-head self-attention (B=1, S=4096, D=512, H=8) on 8 trn2 NeuronCores.

Sharding: one head per core (head/tensor parallel). Each core computes its
head's Q/K/V projections from the full (transposed) query, runs attention
without materializing the full score matrix (streaming over key chunks,
softmax denominator via a ones-column augmented V^T), applies its slice of
out_proj fused with softmax normalization, and writes an unnormalized partial
[S, D] output. Host sums the 8 partials and adds out_proj bias.

All matmul operands are bf16 (f32 PSUM accumulate): the PE streams bf16
moving operands at 1 col/cycle vs ~2 cycles for f32r, and input/output HBM
traffic halves. The softmax scale is folded into wq on the host. Query DMA
is chunked per 512-column group so many DMA engines run in parallel and
projections start before the full load lands. out_proj + normalization +
output DMA for group g are interleaved into group g+1's attention batches.
"""

import sys

sys.path.insert(0, "/opt/trn_rl_repo")

import numpy as np
import ml_dtypes

BF = ml_dtypes.bfloat16

EMBED = 512
HEADS = 8
HD = 64          # head dim
S = 4096         # sequence length
P = 128          # partitions
NSK = S // P     # 32 key chunks of 128
QG = 512         # query group width (matmul free dim)
NQG = S // QG    # 8 query groups
NDC = EMBED // P # 4 contraction chunks for projections
SCALE = HD ** -0.5
EXP_BATCH = 3    # key chunks per exp batch (PSUM banks per slot)

_compiled = {}


def _build(n_cores=8):
    import concourse.bacc as bacc
    import concourse.mybir as mybir
    import concourse.tile as tile

    f32 = mybir.dt.float32
    bf16 = mybir.dt.bfloat16

    nc = bacc.Bacc("TRN2", target_bir_lowering=False, debug=False,
                   num_devices=n_cores)

    qt = nc.dram_tensor("qt", [EMBED, S], bf16, kind="ExternalInput")
    wq = nc.dram_tensor("wq", [EMBED, HD], bf16, kind="ExternalInput")
    wk = nc.dram_tensor("wk", [EMBED, HD], bf16, kind="ExternalInput")
    wv = nc.dram_tensor("wv", [EMBED, HD], bf16, kind="ExternalInput")
    wo = nc.dram_tensor("wo", [HD, EMBED], bf16, kind="ExternalInput")
    bq = nc.dram_tensor("bq", [HD, 1], f32, kind="ExternalInput")
    bk = nc.dram_tensor("bk", [HD, 1], f32, kind="ExternalInput")
    bv = nc.dram_tensor("bv", [P, HD], f32, kind="ExternalInput")
    out_p = nc.dram_tensor("out_p", [S, EMBED], bf16, kind="ExternalOutput")

    with tile.TileContext(nc) as tc:
        _emit(tc, nc, mybir, qt, wq, wk, wv, wo, bq, bk, bv, out_p)

    nc.compile()
    return nc


def _emit(tc, nc, mybir, qt, wq, wk, wv, wo, bq, bk, bv, out_p):
    from contextlib import ExitStack

    f32 = mybir.dt.float32
    bf16 = mybir.dt.bfloat16
    Exp = mybir.ActivationFunctionType.Exp
    Copy = mybir.ActivationFunctionType.Copy

    with ExitStack() as ctx:
        singles = ctx.enter_context(tc.tile_pool(name="singles", bufs=1))

        # --- warm up the ACT exp table while DMAs run ---
        warm = singles.tile([1, 1], f32)
        nc.vector.memset(warm, 0.0)
        warm2 = singles.tile([1, 1], f32)
        nc.scalar.activation(warm2, warm, Exp)

        # --- query load: one DMA per (group, chunk) so 16 DMA engines run
        # in parallel and group 0 lands early. The query stream is the
        # bandwidth-bound prefix, so dispatch group 0 before the weights. ---
        qt_sb = [singles.tile([P, S], bf16, tag=f"qt{c}", name=f"qt_sb{c}")
                 for c in range(NDC)]
        # stationaries zero-padded so every matmul runs as a (128,128) PE
        # tile: avoids the ~130ns reconfig penalty on shape switches
        wq_sb = singles.tile([P, NDC, P], bf16)
        wk_sb = singles.tile([P, NDC, P], bf16)
        wv_sb = singles.tile([P, NDC, HD], bf16)
        nc.vector.memset(wq_sb[:, :, HD:P], 0.0)
        nc.vector.memset(wk_sb[:, :, HD:P], 0.0)
        dq = [nc.sync, nc.gpsimd, nc.scalar]
        di = 0

        def load_qt_group(g):
            nonlocal di
            gsl = slice(g * QG, (g + 1) * QG)
            for c in range(NDC):
                dq[di % 3].dma_start(out=qt_sb[c][:, gsl],
                                     in_=qt[c * P:(c + 1) * P, gsl])
                di += 1

        load_qt_group(0)
        load_qt_group(1)
        for c in range(NDC):
            nc.gpsimd.dma_start(out=wk_sb[:, c, 0:HD],
                                in_=wk[c * P:(c + 1) * P, :])
            nc.scalar.dma_start(out=wq_sb[:, c, 0:HD],
                                in_=wq[c * P:(c + 1) * P, :])
            nc.sync.dma_start(out=wv_sb[:, c, :], in_=wv[c * P:(c + 1) * P, :])
        load_qt_group(2)
        wo_sb = singles.tile([P, EMBED], bf16)
        nc.vector.memset(wo_sb[HD:P, :], 0.0)
        nc.gpsimd.dma_start(out=wo_sb[0:HD, :], in_=wo[:, :])
        bq_sb = singles.tile([HD, 1], f32)
        nc.scalar.dma_start(out=bq_sb, in_=bq[:, :])
        bk_sb = singles.tile([HD, 1], f32)
        nc.gpsimd.dma_start(out=bk_sb, in_=bk[:, :])
        bv_sb = singles.tile([P, HD], f32)
        nc.sync.dma_start(out=bv_sb, in_=bv[:, :])
        for g in range(3, NQG):
            load_qt_group(g)

        # persistent activations; q/k/ot padded with zero rows 64-127 so the
        # attention matmuls all use full-K (128,128) PE tiles
        q_sb = singles.tile([P, S], bf16)       # Q^T (pre-scaled): [hd, s]
        k_sb = singles.tile([P, S], bf16)       # K^T: [hd, s]
        vt_sb = singles.tile([P, NSK, HD + 1], bf16)  # V^T chunks + ones col
        ot_sb = singles.tile([P, S], bf16)      # unnormalized attn out^T
        den_row = singles.tile([1, S], f32)     # denominator, row layout
        den_all = singles.tile([P, NSK], f32)   # denominator, [sq%128, chunk]
        recip_all = singles.tile([P, NSK], f32) # 1/denominator

        nc.vector.memset(q_sb[HD:P, :], 0.0)
        nc.vector.memset(k_sb[HD:P, :], 0.0)
        nc.vector.memset(ot_sb[HD:P, :], 0.0)
        nc.vector.memset(vt_sb[:, :, HD:HD + 1], 1.0)

        # --- stage B: projections ---
        with ExitStack() as bctx:
            pqk = bctx.enter_context(
                tc.tile_pool(name="pqk", bufs=2, space="PSUM"))
            pvp = bctx.enter_context(
                tc.tile_pool(name="pvp", bufs=2, space="PSUM"))

            # per landed query group: K, Q, then V of that group's chunks, so
            # the PE always has work while later groups' DMAs stream in
            for g in range(NQG):
                sl = slice(g * QG, (g + 1) * QG)
                acc_k = pqk.tile([P, QG], f32, tag="pj")
                for c in range(NDC):
                    nc.tensor.matmul(acc_k, wk_sb[:, c, :], qt_sb[c][:, sl],
                                     start=(c == 0), stop=(c == NDC - 1))
                nc.vector.tensor_scalar_add(k_sb[0:HD, sl], acc_k[0:HD, :],
                                            bk_sb)
                acc_q = pqk.tile([P, QG], f32, tag="pj")
                for c in range(NDC):
                    nc.tensor.matmul(acc_q, wq_sb[:, c, :], qt_sb[c][:, sl],
                                     start=(c == 0), stop=(c == NDC - 1))
                nc.vector.tensor_scalar_add(q_sb[0:HD, sl], acc_q[0:HD, :],
                                            bq_sb)
                for i in range(QG // P):
                    s = g * (QG // P) + i
                    ssl = slice(s * P, (s + 1) * P)
                    acc_v = pvp.tile([P, HD], f32, tag="pv")
                    for c in range(NDC):
                        nc.tensor.matmul(acc_v, qt_sb[c][:, ssl],
                                         wv_sb[:, c, :],
                                         start=(c == 0), stop=(c == NDC - 1))
                    nc.vector.tensor_add(vt_sb[:, s, 0:HD], acc_v, bv_sb)

        # --- attention + fused out_proj epilogue ---
        with ExitStack() as cctx:
            s_pool = cctx.enter_context(
                tc.tile_pool(name="s_pool", bufs=2, space="PSUM"))
            acc_pool = cctx.enter_context(
                tc.tile_pool(name="acc_pool", bufs=1, space="PSUM"))
            op_pool = cctx.enter_context(
                tc.tile_pool(name="op_pool", bufs=1, space="PSUM"))
            p_pool = cctx.enter_context(tc.tile_pool(name="p_pool", bufs=4))
            o_pool = cctx.enter_context(tc.tile_pool(name="o_pool", bufs=3))

            oq = [nc.sync, nc.gpsimd]

            def epilogue(g, out_acc, u):
                gsl = slice(g * QG, (g + 1) * QG)
                last = g == NQG - 1
                if u == 0:
                    # evict numerator (bf16 cast) + denominator row; start
                    # the row->column transpose DMAs; reciprocal
                    nc.vector.tensor_copy(ot_sb[0:HD, gsl], out_acc[0:HD, :])
                    nc.vector.tensor_copy(den_row[:, gsl],
                                          out_acc[HD:HD + 1, :])
                    tq = [nc.sync, nc.gpsimd, nc.scalar] if last else oq
                    for i in range(QG // P):
                        j = g * (QG // P) + i
                        tq[i % len(tq)].dma_start(
                            out=den_all[:, j:j + 1],
                            in_=den_row[0:1, j * P:(j + 1) * P])
                    nc.vector.reciprocal(
                        recip_all[:, g * (QG // P):(g + 1) * (QG // P)],
                        den_all[:, g * (QG // P):(g + 1) * (QG // P)])
                else:
                    t = g * (QG // P) + (u - 1)
                    tsl = slice(t * P, (t + 1) * P)
                    if last:
                        # attention is done: the score-pool banks are free,
                        # use them so the four tail matmuls double-buffer
                        o_ps = s_pool.tile([P, EMBED], f32, tag="sps")
                    else:
                        o_ps = op_pool.tile([P, EMBED], f32, tag="op")
                    nc.tensor.matmul(o_ps, ot_sb[:, tsl], wo_sb,
                                     start=True, stop=True)
                    o_sb = o_pool.tile([P, EMBED], bf16, tag="o")
                    if last and u % 2 == 0:
                        # ACT is idle once the final exp is done: normalize
                        # + evict via activation(Copy, scale=1/den)
                        nc.scalar.activation(o_sb, o_ps, Copy,
                                             scale=recip_all[:, t:t + 1])
                    else:
                        nc.vector.tensor_scalar_mul(o_sb, o_ps,
                                                    recip_all[:, t:t + 1])
                    if last:
                        # final group: quarter the writes so 4 DMA engines
                        # drain the tail in parallel
                        for r in range(4):
                            rsl = slice(t * P + r * 32, t * P + (r + 1) * 32)
                            oq[r % 2].dma_start(out=out_p[rsl, :],
                                                in_=o_sb[r * 32:(r + 1) * 32, :])
                    else:
                        oq[t % 2].dma_start(out=out_p[tsl, :], in_=o_sb)

            def batch_list(g):
                if g == NQG - 1:
                    # final group: taper the last batches so the S->exp->AV
                    # drain at kernel end is shorter
                    sizes = [EXP_BATCH] * ((NSK - 2) // EXP_BATCH) + [1, 1]
                else:
                    sizes = [EXP_BATCH] * (NSK // EXP_BATCH)
                    if NSK % EXP_BATCH:
                        sizes.append(NSK % EXP_BATCH)
                out, start = [], 0
                for s in sizes:
                    out.append(list(range(start, start + s)))
                    start += s
                return out

            prev = None  # (g, out_acc) of the previous group
            for g in range(NQG):
                gsl = slice(g * QG, (g + 1) * QG)
                if prev is not None:
                    epilogue(prev[0], prev[1], 0)
                out_acc = acc_pool.tile([HD + 1, QG], f32, tag="acc")
                unit = 1
                batches = batch_list(g)
                n_b = len(batches)
                # paired batches: [S,S][exp,exp][AV,AV] halves the PE's
                # stationary-shape switches (each switch costs ~130ns)
                for b0 in range(0, n_b, 2):
                    pair = []
                    for b in (b0, b0 + 1):
                        if b >= n_b:
                            continue
                        chunks = batches[b]
                        s_ps = s_pool.tile([P, EXP_BATCH * QG], f32,
                                           tag="sps")
                        for i, s in enumerate(chunks):
                            nc.tensor.matmul(
                                s_ps[:, i * QG:(i + 1) * QG],
                                k_sb[:, s * P:(s + 1) * P], q_sb[:, gsl],
                                start=True, stop=True)
                        pair.append((chunks, s_ps))
                    ppair = []
                    for chunks, s_ps in pair:
                        nb = len(chunks)
                        p_sb = p_pool.tile([P, EXP_BATCH * QG], bf16, tag="p")
                        nc.scalar.activation(p_sb[:, :nb * QG],
                                             s_ps[:, :nb * QG], Exp)
                        ppair.append((chunks, p_sb))
                    for chunks, p_sb in ppair:
                        for i, s in enumerate(chunks):
                            nc.tensor.matmul(
                                out_acc, vt_sb[:, s, :],
                                p_sb[:, i * QG:(i + 1) * QG],
                                start=(s == 0), stop=(s == NSK - 1))
                    if prev is not None and unit <= QG // P:
                        epilogue(prev[0], prev[1], unit)
                        unit += 1
                prev = (g, out_acc)

            # final group's epilogue
            for u in range(QG // P + 1):
                epilogue(prev[0], prev[1], u)


def _in_maps(query, in_proj_weight, in_proj_bias, out_proj_weight):
    q2d = np.asarray(query, dtype=np.float32).reshape(S, EMBED)
    qt = np.ascontiguousarray(q2d.T).astype(BF)
    w = np.asarray(in_proj_weight, dtype=np.float32)
    b = np.asarray(in_proj_bias, dtype=np.float32)
    wout = np.asarray(out_proj_weight, dtype=np.float32)
    maps = []
    for h in range(HEADS):
        hs = slice(h * HD, (h + 1) * HD)
        ks = slice(EMBED + h * HD, EMBED + (h + 1) * HD)
        vs = slice(2 * EMBED + h * HD, 2 * EMBED + (h + 1) * HD)
        maps.append({
            "qt": qt,
            # softmax scale folded into the Q projection
            "wq": np.ascontiguousarray(w[hs, :].T * SCALE).astype(BF),
            "wk": np.ascontiguousarray(w[ks, :].T).astype(BF),
            "wv": np.ascontiguousarray(w[vs, :].T).astype(BF),
            "wo": np.ascontiguousarray(wout[:, hs].T).astype(BF),
            "bq": np.ascontiguousarray(
                (b[hs] * SCALE).reshape(HD, 1)).astype(np.float32),
            "bk": np.ascontiguousarray(b[ks].reshape(HD, 1)).astype(np.float32),
            "bv": np.ascontiguousarray(
                np.broadcast_to(b[vs], (P, HD))).astype(np.float32),
        })
    return maps


def get_nc():
    if "nc" not in _compiled:
        _compiled["nc"] = _build()
    return _compiled["nc"]


def kernel(query, in_proj_weight, in_proj_bias, out_proj_weight, out_proj_bias):
    from concourse.bass_utils import run_bass_kernel_spmd

    nc = get_nc()
    maps = _in_maps(query, in_proj_weight, in_proj_bias, out_proj_weight)
    res = run_bass_kernel_spmd(nc, maps, core_ids=list(range(HEADS)))
    acc = np.zeros((S, EMBED), dtype=np.float32)
    for h in range(HEADS):
        acc += np.asarray(res.results[h]["out_p"], dtype=np.float32)
    acc += np.asarray(out_proj_bias, dtype=np.float32)[None, :]
    return acc.reshape(np.asarray(query).shape).astype(np.float32)
